# revision 44
# baseline (speedup 1.0000x reference)
"""AlphaKnotLoss on 8 TRN2 NeuronCores (Bass/Tile, SPMD data-parallel).

Reference computation (B=4096 graphs x 512 nodes x A=10 actions):
  loss_val    = mean((values - target_vals)^2)                  over B
  per graph g: Z[g]   = sum_{n in g, a} exp(logits[n,a])
               Lin[g] = sum_{n in g, a} target_probs[n,a]*logits[n,a]
               V[g]   = sum_{n in g, a} target_probs[n,a]
               lp[g]  = (log(Z[g]+eps) - Lin[g]) / (V[g]+eps)
  out = loss_val + mean(lp)

Sharding: data-parallel over graphs. Each of the 8 cores owns 512 whole
graphs = 262144 nodes. Per-core layout: the (262144, 10) node-major shard
is viewed as (128, 20480) so partition p holds 4 whole graphs
(4 x 512 nodes x 10 actions = 20480 contiguous floats); a graph is a
5120-wide contiguous block of the free axis, split across column tiles.

The host casts logits/probs to bf16 before upload (STREAM_BF16): this
halves the HBM stream (the original binding constraint) to ~10.5 MB/core,
which puts all 8 cores together right at the chip HBM roofline
(~29us/core). All accumulation stays f32, so only the bf16 input rounding
propagates: measured rel err 2.3e-07 (gate is 2e-2).

Per column tile (128 x w): ACT does exp with fused accumulate (Z) plus
copy-accumulates (V) for the leading columns; DVE does one fused
multiply+reduce via scalar_tensor_tensor (Lin) plus reduce_sum (V) for
the V_TAIL_DVE trailing columns. With bf16 the two engines are jointly
reduction-bound (exp + V + Lin = 3 passes over the data at ~1.07 ns/elem,
~33us per engine) and finish just after the stream. The last graph tapers
[2048, 1536, 1024, 512] so the post-last-byte serial chain is thin.

Epilogue: per-graph losses on (128, GPP) stats accumulate into S[128, 8];
with DIRECT_OUT the per-partition partials DMA straight out and the host
reduces (skips the PE ones-matmul + PSUM copy + extra descriptor gen).
Mode "partials" (default) avoids any collective barrier entirely.
"""

import numpy as np

B = 4096
NPG = 512
N = B * NPG
A = 10
EPS = 1e-9
M = 8  # cores

NC_NODES = N // M          # 262144 nodes per core
NC_GRAPHS = B // M         # 512 graphs per core
P = 128                    # SBUF partitions
FREE = NC_NODES * A // P   # 20480 f32 per partition
GPP = NC_GRAPHS // P       # 4 graphs per partition
GFREE = NPG * A            # 5120 f32 per graph

# Per-graph tile widths (must sum to GFREE). A thin final slice keeps the
# critical path after the last DMA byte short.
GRAPH_TILES = [2560, 2560]
TPG = len(GRAPH_TILES)
NT = GPP * TPG
# When True, the very last graph column uses widths LAST_WIDTHS so the
# final DVE/ACT ops after the last DMA byte are thin (geometric taper:
# each op after the last byte processes only the final sliver).
SPLIT_LAST = True
LAST_WIDTHS = [2048, 1536, 1024, 512]

IO_BUFS = 4
WORK_BUFS = 3

# Number of trailing columns whose V-reduce runs on DVE (reduce_sum)
# instead of ACT (Copy+accum) — balances the two engines now that bf16
# streaming makes ACT the near-bottleneck (bf16 sweep: 5 beats 2/4/7/9).
V_TAIL_DVE = 5

# When True, invert the V split: DVE reduces the FIRST V_TAIL_DVE columns
# (where it has arrival slack) and ACT copy-accumulates the tail, so the
# DVE queue is not double-loaded right at stream end.
V_FRONT = False

# Reorder the stream tail: the last graph's two logits half-tiles are
# DMA'd before the matching probs half-tiles, so exp + the Ln table load
# + LN finish before the last probs byte lands.
TAIL_REORDER = False

# Engine whose HWDGE queue issues the probs-tile DMAs ("sync" = same queue
# as logits; "tensor" = the otherwise-idle PE sequencer, doubling the
# DMA-issue paths).
PT_ENGINE = "sync"

# Columns whose DMA is issued as [0:120] + [120:128] row-splits. A sync-HWDGE
# transfer deals ceil(nrows/16) contiguous rows per SDMA engine starting at
# engine 0, so the [120,*] part leaves engine 15 idle and the [8,*] part rides
# engines 0-7. SDMA engine 15 is ~20% slower on core 0 whenever the profiler
# is attached (the slow mode of the bimodal exec distribution); skipping it
# for the last graph's columns drops e15 to ~75% of the per-engine byte load
# so its backlog drains before the other engines finish. A/B (7 interleaved
# reps each): split 81.0-87.0us, never fast; baseline 68.7-81.7 — the extra
# transfers slow the whole stream more than the e15 skip helps. Keep False.
E15_SPLIT_TAIL = False

# Number of leading columns whose DMAs issue from the scalar (Act) HWDGE
# instead of sync: the Act sequencer's preamble runs in parallel with
# Sync's, so the first descriptors generate ~1.5us earlier.
FIRST_SCALAR_COLS = 0

# Band-skew layout: rows 120-127 hold only 2 graphs, so the columns of graph
# slots 2 and 3 are [0:120]-row transfers that put ZERO descriptors on SDMA
# engine 15 (sync HWDGE deals ceil(nrows/16) contiguous rows per engine from
# engine 0; engine 15 is ~20% slower on core 0 whenever the NTFF profiler
# runs — the slow mode of the bimodal exec distribution). The 16 displaced
# graphs ride as separate [16, 5120] band tensors — ONE graph per partition
# row, ONE 20.5KB descriptor per SDMA engine (the per-descriptor fixed cost
# is ~300ns, so band shape must minimize descriptor count, which is what
# sank the [128, 320] variant). Two PE matmuls against a host-provided
# selection matrix W[16, 128] scatter the band's per-graph sums onto rows
# 120-127, into two extra stats columns that the slot-2/slot-3 reduces fold
# in with no extra tail ops. Engine 15 carries ~52% of the other engines'
# bytes, so it never gates the stream even when 25% slow.
BAND_SKEW = False
SKEW_LO = 120       # rows >= SKEW_LO carry 2 graphs under BAND_SKEW
NBAND = 16          # displaced graphs (one band-tensor row each)

# False: partials leave via a PE ones-matmul -> PSUM[1,8] -> SBUF -> DRAM.
# True: DMA the per-partition S[128,8] straight out; host reduces. With the
# bf16 tapered tail this skips the matmul+PSUM-copy+descriptor-gen chain:
# A/B 59.9 vs 60.4 / 51.5 vs 52+ us. Keep True.
DIRECT_OUT = True

# "lfirst": stream the whole logits shard first (kept resident in SBUF,
# exp/log(Z) finish mid-stream), then stream probs; the post-last-byte
# chain shrinks to one half-width stt + copy + the scalar epilogue.
LFIRST = False

# Stream logits/probs as bf16 (host casts before upload): halves the HBM
# bytes of the DMA-bound stream and doubles DVE elementwise throughput.
# All accumulation stays f32 (ACT/DVE accumulators), so only the 0.2%
# input rounding propagates: measured rel err ~1e-4 vs the 2e-2 gate.
# vals/tvals stay f32 (tiny, and keeps loss_val exact).
STREAM_BF16 = True


def set_params(graph_tiles=None, io_bufs=None, work_bufs=None, mode=None,
               split_last=None, lfirst=None, direct_out=None,
               pt_engine=None, tail_reorder=None, v_tail_dve=None,
               e15_split_tail=None, first_scalar_cols=None, band_skew=None,
               stream_bf16=None, last_widths=None, v_front=None):
    """Tweak build knobs (test harness only); clears the build cache."""
    global GRAPH_TILES, TPG, NT, IO_BUFS, WORK_BUFS, MODE, SPLIT_LAST, LFIRST
    global DIRECT_OUT, PT_ENGINE, TAIL_REORDER, V_TAIL_DVE, E15_SPLIT_TAIL
    global FIRST_SCALAR_COLS, BAND_SKEW, STREAM_BF16, LAST_WIDTHS, V_FRONT
    if v_front is not None:
        V_FRONT = v_front
    if last_widths is not None:
        assert sum(last_widths) == GFREE
        LAST_WIDTHS = list(last_widths)
    if stream_bf16 is not None:
        STREAM_BF16 = stream_bf16
    if band_skew is not None:
        BAND_SKEW = band_skew
    if first_scalar_cols is not None:
        FIRST_SCALAR_COLS = first_scalar_cols
    if e15_split_tail is not None:
        E15_SPLIT_TAIL = e15_split_tail
    if v_tail_dve is not None:
        V_TAIL_DVE = v_tail_dve
    if tail_reorder is not None:
        TAIL_REORDER = tail_reorder
    if pt_engine is not None:
        PT_ENGINE = pt_engine
    if split_last is not None:
        SPLIT_LAST = split_last
    if lfirst is not None:
        LFIRST = lfirst
    if direct_out is not None:
        DIRECT_OUT = direct_out
    if graph_tiles is not None:
        assert sum(graph_tiles) == GFREE
        GRAPH_TILES = list(graph_tiles)
        TPG = len(GRAPH_TILES)
        NT = GPP * TPG
    if io_bufs is not None:
        IO_BUFS = io_bufs
    if work_bufs is not None:
        WORK_BUFS = work_bufs
    if mode is not None:
        MODE = mode
    _CACHE.clear()

MODE = "partials"

_CACHE = {}


def _build(mode):
    import concourse.bacc as bacc
    import concourse.mybir as mybir
    import concourse.tile as tile

    f32 = mybir.dt.float32
    sdt = mybir.dt.bfloat16 if STREAM_BF16 else f32
    Alu = mybir.AluOpType
    Act = mybir.ActivationFunctionType
    AX = mybir.AxisListType.X

    nc = bacc.Bacc("TRN2", target_bir_lowering=False, debug=False,
                   num_devices=M)

    logits = nc.dram_tensor("logits", [P, FREE], sdt, kind="ExternalInput")
    probs = nc.dram_tensor("probs", [P, FREE], sdt, kind="ExternalInput")
    vals = nc.dram_tensor("vals", [P, GPP], f32, kind="ExternalInput")
    tvals = nc.dram_tensor("tvals", [P, GPP], f32, kind="ExternalInput")
    if BAND_SKEW:
        lband = nc.dram_tensor("lband", [NBAND, GFREE], sdt,
                               kind="ExternalInput")
        pband = nc.dram_tensor("pband", [NBAND, GFREE], sdt,
                               kind="ExternalInput")
        wsel = nc.dram_tensor("wsel", [NBAND, 2 * P], f32,
                              kind="ExternalInput")
    if mode == "allreduce" or not DIRECT_OUT:
        out = nc.dram_tensor("out", [1, 8], f32, kind="ExternalOutput")
    else:
        # per-partition partials go out directly; host does the final
        # 128x2-per-core reduction (no PE/PSUM in the graph at all)
        out = nc.dram_tensor("out", [P, 8], f32, kind="ExternalOutput")

    import contextlib

    with tile.TileContext(nc) as tc:
        with contextlib.ExitStack() as ctx:
            iop = ctx.enter_context(tc.tile_pool(name="io", bufs=IO_BUFS))
            wp = ctx.enter_context(tc.tile_pool(name="work", bufs=WORK_BUFS))
            sp = ctx.enter_context(tc.tile_pool(name="stats", bufs=1))
            if BAND_SKEW:
                bpool = ctx.enter_context(tc.tile_pool(name="band", bufs=1))
            if mode == "allreduce" or not DIRECT_OUT:
                pp = ctx.enter_context(
                    tc.tile_pool(name="psum", bufs=1, space="PSUM"))
            if mode == "allreduce":
                dp = ctx.enter_context(
                    tc.tile_pool(name="dram", bufs=1, space="DRAM"))
            # per-tile widths for each graph column; optionally split the
            # tail of the last graph for a shorter post-DMA chain
            widths = []
            for g in range(GPP):
                if SPLIT_LAST and g == GPP - 1:
                    widths += list(LAST_WIDTHS)
                else:
                    widths += list(GRAPH_TILES)
            ncols = len(widths)
            # Stats columns under BAND_SKEW: data cols for graph slots 0-2
            # keep their index, slot 3's shift up by one; two band columns
            # (slot-2 sums at index 2*TPG+2, slot-3 sums at the end) are
            # filled from PSUM mid-stream so the slot reduces fold them in.
            if BAND_SKEW:
                assert SPLIT_LAST and GPP == 4 and TPG == 2
                bcol2 = 2 * TPG + 2                     # 6
                colmap = [0, 1, 2, 3, 4, 5, 7, 8, 9]
                ncols_b = ncols + 2                     # 11; bcol3 = 10
                bcol3 = ncols_b - 1
            else:
                colmap = list(range(ncols))
                ncols_b = ncols
            Z = sp.tile([P, ncols_b], f32)
            Lin = sp.tile([P, ncols_b], f32)
            V = sp.tile([P, ncols_b], f32)

            fmax = max(GRAPH_TILES)
            offs = []
            o = 0
            for w in widths:
                offs.append(o)
                o += w

            # columns belonging to the last graph get engine-15-free DMAs
            e15_free = set(range(ncols - (TPG + (1 if SPLIT_LAST else 0)),
                                 ncols)) if E15_SPLIT_TAIL else set()
            # under BAND_SKEW, rows 120-127 have no data for graph slots
            # 2 and 3: transfer rows [0:120] only (zero bytes on SDMA
            # engine 15) and memset rows [96:128] so the full-partition
            # compute reads neutral values there
            band_cols = (set(j for j in range(ncols)
                             if offs[j] >= (GPP - 2) * GFREE)
                         if BAND_SKEW else set())

            def col_dma(dst, src_t, j, w, fill=0.0):
                eng = nc.scalar if j < FIRST_SCALAR_COLS else nc.sync
                if j in band_cols:
                    nc.gpsimd.memset(dst[96:128, :w], fill)
                    eng.dma_start(dst[0:SKEW_LO, :w],
                                  src_t[0:SKEW_LO, offs[j]:offs[j] + w])
                elif j in e15_free:
                    eng.dma_start(dst[0:120, :w],
                                  src_t[0:120, offs[j]:offs[j] + w])
                    eng.dma_start(dst[120:128, :w],
                                  src_t[120:128, offs[j]:offs[j] + w])
                else:
                    eng.dma_start(dst[:, :w],
                                  src_t[:, offs[j]:offs[j] + w])

            def emit_lt(j):
                lt = iop.tile([P, fmax], sdt, tag="lt", name=f"lt{j}")
                col_dma(lt, logits, j, widths[j], fill=-80.0)
                return lt

            # op outputs are dummies (only accum_out matters) — under
            # BAND_SKEW share one per engine to make SBUF room for the
            # band tiles (WAW ordering matches engine program order)
            if BAND_SKEW:
                act_dum = wp.tile([P, fmax], sdt, tag="actd", name="actd")
                dve_dum = wp.tile([P, fmax], sdt, tag="dved", name="dved")

            def emit_exp(j, lt):
                w = widths[j]
                c = colmap[j]
                et = (act_dum if BAND_SKEW else
                      wp.tile([P, fmax], sdt, tag="et", name=f"et{j}"))
                nc.scalar.activation(et[:, :w], lt[:, :w], Act.Exp,
                                     accum_out=Z[:, c:c + 1])

            def emit_pt_side(j, lt):
                w = widths[j]
                c = colmap[j]
                pt = iop.tile([P, fmax], sdt, tag="pt", name=f"pt{j}")
                col_dma(pt, probs, j, w)
                on_dve = (j < V_TAIL_DVE) if V_FRONT else (
                    j >= ncols - V_TAIL_DVE)
                if on_dve:
                    nc.vector.reduce_sum(V[:, c:c + 1], pt[:, :w], axis=AX)
                else:
                    cp = (act_dum if BAND_SKEW else
                          wp.tile([P, fmax], sdt, tag="cp", name=f"cp{j}"))
                    nc.scalar.activation(cp[:, :w], pt[:, :w], Act.Copy,
                                         accum_out=V[:, c:c + 1])
                prod = (dve_dum if BAND_SKEW else
                        wp.tile([P, fmax], sdt, tag="prod", name=f"prod{j}"))
                nc.vector.scalar_tensor_tensor(
                    out=prod[:, :w], in0=lt[:, :w], scalar=1.0,
                    in1=pt[:, :w], op0=Alu.mult, op1=Alu.mult,
                    accum_out=Lin[:, c:c + 1])

            split_tail = SPLIT_LAST or BAND_SKEW
            nuni = (GPP - 1) * TPG if split_tail else GPP * TPG
            Zg = sp.tile([P, GPP], f32)
            Lg = sp.tile([P, GPP], f32)
            Vg = sp.tile([P, GPP], f32)
            zp = sp.tile([P, GPP], f32)
            logz = sp.tile([P, GPP], f32)

            def emit_logz():
                if BAND_SKEW:
                    nc.vector.reduce_sum(
                        Zg[:, 0:2],
                        Z[:, 0:4].rearrange("p (g t) -> p g t", t=TPG),
                        axis=AX)
                    nc.vector.reduce_sum(
                        Zg[:, 2:3], Z[:, 4:bcol2 + 1], axis=AX)
                    nc.vector.reduce_sum(
                        Zg[:, 3:4], Z[:, bcol2 + 1:ncols_b], axis=AX)
                elif split_tail:
                    nc.vector.reduce_sum(
                        Zg[:, 0:GPP - 1],
                        Z[:, 0:nuni].rearrange("p (g t) -> p g t", t=TPG),
                        axis=AX)
                    nc.vector.reduce_sum(
                        Zg[:, GPP - 1:GPP], Z[:, nuni:ncols_b], axis=AX)
                else:
                    nc.vector.reduce_sum(
                        Zg[:, :],
                        Z[:, :].rearrange("p (g t) -> p g t", t=TPG),
                        axis=AX)
                nc.vector.tensor_scalar_add(zp[:, :], Zg[:, :], EPS)
                nc.scalar.activation(logz[:, :], zp[:, :], Act.Ln)

            def emit_band():
                # host-filled selection pair: cols 0-127 select band graphs
                # 0-7 (slot 2 of rows 120-127), cols 128-255 graphs 8-15
                # (slot 3); zero rows kill the other group, keeping both
                # matmuls at partition base 0
                wt = sp.tile([NBAND, 2 * P], f32)
                nc.sync.dma_start(wt[:, :], wsel[:, :])
                bl = bpool.tile([NBAND, GFREE], f32, tag="bl", name="bl")
                nc.sync.dma_start(bl[:, :], lband[:, :])
                bp = bpool.tile([NBAND, GFREE], f32, tag="bp", name="bp")
                nc.sync.dma_start(bp[:, :], pband[:, :])
                BS = sp.tile([NBAND, 3], f32)
                bdum = bpool.tile([NBAND, GFREE], f32, tag="bdum",
                                  name="bdum")
                nc.scalar.activation(bdum[:, :], bl[:, :], Act.Exp,
                                     accum_out=BS[:, 0:1])
                nc.scalar.activation(bdum[:, :], bp[:, :], Act.Copy,
                                     accum_out=BS[:, 1:2])
                nc.vector.scalar_tensor_tensor(
                    out=bdum[:, :], in0=bl[:, :], scalar=1.0, in1=bp[:, :],
                    op0=Alu.mult, op1=Alu.mult, accum_out=BS[:, 2:3])
                # per-graph scatter onto rows 120-127: cols 0-2 = slot-2
                # band sums (Z, V, Lin), cols 3-5 = slot-3
                pf = pp.tile([P, 6], f32)
                nc.tensor.matmul(pf[:, 0:3], wt[:, 0:P], BS[:, :],
                                 start=True, stop=True)
                nc.tensor.matmul(pf[:, 3:6], wt[:, P:2 * P], BS[:, :],
                                 start=True, stop=True)
                # rows 0-119 of pf are zero, so the copies only affect the
                # skew rows' stat slots
                nc.vector.tensor_copy(Z[:, bcol2:bcol2 + 1], pf[:, 0:1])
                nc.vector.tensor_copy(V[:, bcol2:bcol2 + 1], pf[:, 1:2])
                nc.vector.tensor_copy(Lin[:, bcol2:bcol2 + 1], pf[:, 2:3])
                nc.vector.tensor_copy(Z[:, bcol3:bcol3 + 1], pf[:, 3:4])
                nc.vector.tensor_copy(V[:, bcol3:bcol3 + 1], pf[:, 4:5])
                nc.vector.tensor_copy(Lin[:, bcol3:bcol3 + 1], pf[:, 5:6])

            late = 2 if (TAIL_REORDER and SPLIT_LAST) else 0
            lts = {}
            for j in range(ncols - late):
                lts[j] = emit_lt(j)
                emit_exp(j, lts[j])
                emit_pt_side(j, lts[j])
                if BAND_SKEW and j == 0:
                    emit_band()
            if late:
                for j in range(ncols - late, ncols):
                    lts[j] = emit_lt(j)
                    emit_exp(j, lts[j])
                emit_logz()
                for j in range(ncols - late, ncols):
                    emit_pt_side(j, lts[j])
            else:
                emit_logz()

            # remaining per-graph sums
            for src, dst in ((Lin, Lg), (V, Vg)):
                if BAND_SKEW:
                    nc.vector.reduce_sum(
                        dst[:, 0:2],
                        src[:, 0:4].rearrange("p (g t) -> p g t", t=TPG),
                        axis=AX)
                    nc.vector.reduce_sum(
                        dst[:, 2:3], src[:, 4:bcol2 + 1], axis=AX)
                    nc.vector.reduce_sum(
                        dst[:, 3:4], src[:, bcol2 + 1:ncols_b], axis=AX)
                elif split_tail:
                    nc.vector.reduce_sum(
                        dst[:, 0:GPP - 1],
                        src[:, 0:nuni].rearrange("p (g t) -> p g t", t=TPG),
                        axis=AX)
                    nc.vector.reduce_sum(
                        dst[:, GPP - 1:GPP], src[:, nuni:ncols_b], axis=AX)
                else:
                    nc.vector.reduce_sum(
                        dst[:, :],
                        src[:, :].rearrange("p (g t) -> p g t", t=TPG),
                        axis=AX)

            num = sp.tile([P, GPP], f32)
            nc.vector.tensor_sub(num[:, :], logz[:, :], Lg[:, :])
            den = sp.tile([P, GPP], f32)
            nc.vector.tensor_scalar_add(den[:, :], Vg[:, :], EPS)
            rec = sp.tile([P, GPP], f32)
            nc.vector.reciprocal(rec[:, :], den[:, :])

            # S[:,0] = per-partition policy sum, S[:,1] = value-sq sum
            S = sp.tile([P, 8], f32)
            nc.gpsimd.memset(S[:, :], 0.0)
            lp = sp.tile([P, GPP], f32)
            nc.vector.scalar_tensor_tensor(
                out=lp[:, :], in0=num[:, :], scalar=1.0, in1=rec[:, :],
                op0=Alu.mult, op1=Alu.mult, accum_out=S[:, 0:1])

            vt = sp.tile([P, GPP], f32)
            tt = sp.tile([P, GPP], f32)
            nc.sync.dma_start(vt[:, :], vals[:, :])
            nc.sync.dma_start(tt[:, :], tvals[:, :])
            d = sp.tile([P, GPP], f32)
            nc.vector.tensor_sub(d[:, :], vt[:, :], tt[:, :])
            d2 = sp.tile([P, GPP], f32)
            nc.vector.scalar_tensor_tensor(
                out=d2[:, :], in0=d[:, :], scalar=1.0, in1=d[:, :],
                op0=Alu.mult, op1=Alu.mult, accum_out=S[:, 1:2])

            if mode == "allreduce" or not DIRECT_OUT:
                # cross-partition sum via matmul with a ones vector
                ones = sp.tile([P, 1], f32)
                nc.gpsimd.memset(ones[:, :], 1.0)
                ps = pp.tile([1, 8], f32)
                nc.tensor.matmul(ps[:, :], ones[:, :], S[:, :],
                                 start=True, stop=True)
                red = sp.tile([1, 8], f32)
                nc.vector.tensor_copy(red[:, :], ps[:, :])
            if mode == "allreduce":
                cin = dp.tile([1, 8], f32)
                cout = dp.tile([1, 8], f32)
                nc.sync.dma_start(cin[:, :], red[:, :])
                nc.gpsimd.collective_compute(
                    "AllReduce", Alu.add,
                    replica_groups=[list(range(M))],
                    ins=[cin[:, :].opt()],
                    outs=[cout[:, :].opt()])
                red2 = sp.tile([1, 8], f32)
                nc.sync.dma_start(red2[:, :], cout[:, :])
                # out = (sum_policy + sum_val) / B
                dummy = sp.tile([1, 2], f32)
                fin = sp.tile([1, 8], f32)
                nc.gpsimd.memset(fin[:, :], 0.0)
                nc.scalar.activation(dummy[:, :], red2[:, 0:2], Act.Copy,
                                     scale=1.0 / B, accum_out=fin[:, 0:1])
                nc.sync.dma_start(out[:, :], fin[:, :])
            elif DIRECT_OUT:
                nc.sync.dma_start(out[:, :], S[:, :])
            else:
                nc.sync.dma_start(out[:, :], red[:, :])

    nc.compile()
    return nc


def _build_lfirst(mode):
    import concourse.bacc as bacc
    import concourse.mybir as mybir
    import concourse.tile as tile

    f32 = mybir.dt.float32
    Alu = mybir.AluOpType
    Act = mybir.ActivationFunctionType
    AX = mybir.AxisListType.X

    nc = bacc.Bacc("TRN2", target_bir_lowering=False, debug=False,
                   num_devices=M)

    logits = nc.dram_tensor("logits", [P, FREE], f32, kind="ExternalInput")
    probs = nc.dram_tensor("probs", [P, FREE], f32, kind="ExternalInput")
    vals = nc.dram_tensor("vals", [P, GPP], f32, kind="ExternalInput")
    tvals = nc.dram_tensor("tvals", [P, GPP], f32, kind="ExternalInput")
    out = nc.dram_tensor("out", [1, 8], f32, kind="ExternalOutput")

    LW = 2560                    # logits tile width
    LNT = FREE // LW             # 8 resident logits tiles
    # probs widths: uniform except the last graph's tail is split in half
    pw = [LW] * (LNT - 1) + [LW // 2, LW // 2]
    PNT = len(pw)                # 9
    LEAD = 3                     # logits tiles ahead of probs in the stream

    with tile.TileContext(nc) as tc:
        with (
            tc.tile_pool(name="lres", bufs=LNT) as lrp,
            tc.tile_pool(name="pio", bufs=IO_BUFS) as pip_,
            tc.tile_pool(name="work", bufs=WORK_BUFS) as wp,
            tc.tile_pool(name="stats", bufs=1) as sp,
            tc.tile_pool(name="psum", bufs=1, space="PSUM") as pp,
        ):
            Z = sp.tile([P, LNT], f32)
            V = sp.tile([P, PNT], f32)
            Lin = sp.tile([P, PNT], f32)

            ltiles = [lrp.tile([P, LW], f32, tag="lt", name=f"lt{j}")
                      for j in range(LNT)]

            def emit_logits(j):
                nc.sync.dma_start(ltiles[j][:, :],
                                  logits[:, j * LW:(j + 1) * LW])
                et = wp.tile([P, LW], f32, tag="et", name=f"et{j}")
                nc.scalar.activation(et[:, :], ltiles[j][:, :], Act.Exp,
                                     accum_out=Z[:, j:j + 1])

            poff = [0]

            def emit_probs(j):
                w = pw[j]
                off = poff[0]
                pt = pip_.tile([P, LW], f32, tag="pt", name=f"pt{j}")
                nc.sync.dma_start(pt[:, :w], probs[:, off:off + w])
                lsrc = ltiles[off // LW][:, off % LW:off % LW + w]
                cp = wp.tile([P, LW], f32, tag="cp", name=f"cp{j}")
                nc.scalar.activation(cp[:, :w], pt[:, :w], Act.Copy,
                                     accum_out=V[:, j:j + 1])
                prod = wp.tile([P, LW], f32, tag="prod", name=f"prod{j}")
                nc.vector.scalar_tensor_tensor(
                    out=prod[:, :w], in0=lsrc, scalar=1.0, in1=pt[:, :w],
                    op0=Alu.mult, op1=Alu.mult,
                    accum_out=Lin[:, j:j + 1])
                poff[0] += w

            # interleaved stream: logits LEAD tiles ahead so exp/log(Z)
            # finish before the probs stream ends
            li = pi = 0
            for j in range(LEAD):
                emit_logits(li)
                li += 1
            while li < LNT:
                emit_probs(pi)
                pi += 1
                emit_logits(li)
                li += 1
            # log(Z+eps) per graph — scheduled right after the last exp,
            # well before the stream ends
            Zg = sp.tile([P, GPP], f32)
            nc.vector.reduce_sum(
                Zg[:, :], Z[:, :].rearrange("p (g t) -> p g t", t=2),
                axis=AX)
            zp = sp.tile([P, GPP], f32)
            nc.vector.tensor_scalar_add(zp[:, :], Zg[:, :], EPS)
            logz = sp.tile([P, GPP], f32)
            nc.scalar.activation(logz[:, :], zp[:, :], Act.Ln)
            while pi < PNT:
                emit_probs(pi)
                pi += 1

            # per-graph sums: graphs 0..2 from column pairs, graph 3 from
            # the last three columns
            Vg = sp.tile([P, GPP], f32)
            Lg = sp.tile([P, GPP], f32)
            for src, dst in ((V, Vg), (Lin, Lg)):
                nc.vector.reduce_sum(
                    dst[:, 0:GPP - 1],
                    src[:, 0:2 * (GPP - 1)].rearrange(
                        "p (g t) -> p g t", t=2),
                    axis=AX)
                nc.vector.reduce_sum(dst[:, GPP - 1:GPP],
                                     src[:, 2 * (GPP - 1):PNT], axis=AX)

            den = sp.tile([P, GPP], f32)
            nc.vector.tensor_scalar_add(den[:, :], Vg[:, :], EPS)
            rec = sp.tile([P, GPP], f32)
            nc.vector.reciprocal(rec[:, :], den[:, :])
            num = sp.tile([P, GPP], f32)
            nc.vector.tensor_sub(num[:, :], logz[:, :], Lg[:, :])

            S = sp.tile([P, 8], f32)
            nc.gpsimd.memset(S[:, :], 0.0)
            lp = sp.tile([P, GPP], f32)
            nc.vector.scalar_tensor_tensor(
                out=lp[:, :], in0=num[:, :], scalar=1.0, in1=rec[:, :],
                op0=Alu.mult, op1=Alu.mult, accum_out=S[:, 0:1])

            vt = sp.tile([P, GPP], f32)
            tt = sp.tile([P, GPP], f32)
            nc.sync.dma_start(vt[:, :], vals[:, :])
            nc.sync.dma_start(tt[:, :], tvals[:, :])
            d = sp.tile([P, GPP], f32)
            nc.vector.tensor_sub(d[:, :], vt[:, :], tt[:, :])
            d2 = sp.tile([P, GPP], f32)
            nc.vector.scalar_tensor_tensor(
                out=d2[:, :], in0=d[:, :], scalar=1.0, in1=d[:, :],
                op0=Alu.mult, op1=Alu.mult, accum_out=S[:, 1:2])

            ones = sp.tile([P, 1], f32)
            nc.gpsimd.memset(ones[:, :], 1.0)
            ps = pp.tile([1, 8], f32)
            nc.tensor.matmul(ps[:, :], ones[:, :], S[:, :],
                             start=True, stop=True)
            red = sp.tile([1, 8], f32)
            nc.vector.tensor_copy(red[:, :], ps[:, :])
            nc.sync.dma_start(out[:, :], red[:, :])

    nc.compile()
    return nc


def _get(mode):
    if mode not in _CACHE:
        _CACHE[mode] = (_build_lfirst(mode) if LFIRST and mode == "partials"
                        else _build(mode))
    return _CACHE[mode]


def _band_gidx():
    """graph index for (row, slot) under BAND_SKEW: rows 0-119 hold graphs
    4p..4p+3; rows 120-127 hold graphs 480+2r, 481+2r plus band graphs
    496+r (slot 2) and 504+r (slot 3)."""
    gidx = np.empty((P, GPP), np.int64)
    for p_ in range(SKEW_LO):
        gidx[p_] = np.arange(4 * p_, 4 * p_ + 4)
    for r in range(P - SKEW_LO):
        gidx[SKEW_LO + r, 0:2] = 480 + 2 * r + np.arange(2)
        gidx[SKEW_LO + r, 2] = 496 + r
        gidx[SKEW_LO + r, 3] = 504 + r
    return gidx


def _band_wsel():
    w = np.zeros((NBAND, 2 * P), np.float32)
    for g in range(8):
        w[g, SKEW_LO + g] = 1.0               # slot-2 selector
        w[8 + g, P + SKEW_LO + g] = 1.0       # slot-3 selector
    return w


def _band_shard(flat_core):
    """(NC_NODES*A,) per-core flat stream -> main [P, FREE] + band
    [NBAND, GFREE]."""
    graphs = flat_core.reshape(NC_GRAPHS, GFREE)
    main = np.zeros((P, FREE), np.float32)
    main[:SKEW_LO] = graphs[:480].reshape(SKEW_LO, 4 * GFREE)
    main[SKEW_LO:, :2 * GFREE] = graphs[480:496].reshape(8, 2 * GFREE)
    band = np.ascontiguousarray(graphs[496:])
    return main, band


def _stream_cast(a):
    if not STREAM_BF16:
        return np.ascontiguousarray(a)
    import ml_dtypes
    return np.ascontiguousarray(a.astype(ml_dtypes.bfloat16))


def _make_in_maps(logits, values, target_probs, target_vals):
    in_maps = []
    if BAND_SKEW:
        gidx = _band_gidx()
        wsel = _band_wsel()
        lg = logits.reshape(M, NC_NODES * A)
        pg = target_probs.reshape(M, NC_NODES * A)
        vg = values.reshape(M, NC_GRAPHS)
        tg = target_vals.reshape(M, NC_GRAPHS)
        for c in range(M):
            lmain, lb = _band_shard(lg[c])
            pmain, pb = _band_shard(pg[c])
            in_maps.append({
                "logits": _stream_cast(lmain),
                "probs": _stream_cast(pmain),
                "lband": _stream_cast(lb),
                "pband": _stream_cast(pb),
                "vals": np.ascontiguousarray(vg[c][gidx].astype(np.float32)),
                "tvals": np.ascontiguousarray(tg[c][gidx].astype(np.float32)),
                "wsel": wsel,
            })
        return in_maps
    lg = logits.reshape(M, P, FREE)
    pg = target_probs.reshape(M, P, FREE)
    vg = values.reshape(M, P, GPP)
    tg = target_vals.reshape(M, P, GPP)
    for c in range(M):
        in_maps.append({
            "logits": _stream_cast(lg[c]),
            "probs": _stream_cast(pg[c]),
            "vals": np.ascontiguousarray(vg[c]),
            "tvals": np.ascontiguousarray(tg[c]),
        })
    return in_maps


def _finalize(mode, results):
    if mode == "allreduce":
        return np.float32(results[0]["out"][0, 0])
    parts = np.stack([r["out"] for r in results])  # (M, P or 1, 8)
    tot = parts.sum(axis=(0, 1), dtype=np.float64)
    return np.float32((tot[0] + tot[1]) / B)


def kernel(logits, values, target_probs, target_vals, batch_counts):
    from concourse import bass_utils

    global STREAM_BF16
    if STREAM_BF16:
        try:
            import ml_dtypes  # noqa: F401
        except ImportError:
            STREAM_BF16 = False
            _CACHE.clear()

    logits = np.asarray(logits, dtype=np.float32)
    values = np.asarray(values, dtype=np.float32)
    target_probs = np.asarray(target_probs, dtype=np.float32)
    target_vals = np.asarray(target_vals, dtype=np.float32)
    batch_counts = np.asarray(batch_counts)

    if not (batch_counts.shape == (B,) and np.all(batch_counts == NPG)):
        # Non-uniform segments never occur for this problem's inputs;
        # numpy fallback keeps the contract total.
        return _kernel_numpy(logits, values, target_probs, target_vals,
                             batch_counts)

    nc = _get(MODE)
    in_maps = _make_in_maps(logits, values, target_probs, target_vals)
    last_err = None
    for _ in range(3):
        try:
            res = bass_utils.run_bass_kernel_spmd(
                nc, in_maps, core_ids=list(range(M)))
            return _finalize(MODE, res.results)
        except Exception as e:  # transient runtime/worker hiccup
            last_err = e
    import sys
    print(f"kernel: device run failed ({last_err}); numpy fallback",
          file=sys.stderr)
    return _kernel_numpy(logits, values, target_probs, target_vals,
                         batch_counts)


def _kernel_numpy(logits, values, target_probs, target_vals, batch_counts):
    counts = batch_counts.astype(np.int64)
    b = counts.shape[0]
    idx = np.repeat(np.arange(b), counts)
    loss_val = np.mean((values - target_vals) ** 2, dtype=np.float32)
    probs_sum = target_probs.sum(axis=1)
    lin = (target_probs * logits).sum(axis=1)
    ex = np.exp(logits).sum(axis=1)
    vc = np.zeros(b, np.float32)
    lg = np.zeros(b, np.float32)
    zg = np.zeros(b, np.float32)
    np.add.at(vc, idx, probs_sum)
    np.add.at(lg, idx, lin)
    np.add.at(zg, idx, ex)
    lp = (np.log(zg + EPS) - lg) / (vc + EPS)
    return np.float32(loss_val + lp.mean())



# revision 45
# speedup vs baseline: 1.1143x; 1.1143x over previous
"""AlphaKnotLoss on 8 TRN2 NeuronCores (Bass/Tile, SPMD data-parallel).

Reference computation (B=4096 graphs x 512 nodes x A=10 actions):
  loss_val    = mean((values - target_vals)^2)                  over B
  per graph g: Z[g]   = sum_{n in g, a} exp(logits[n,a])
               Lin[g] = sum_{n in g, a} target_probs[n,a]*logits[n,a]
               V[g]   = sum_{n in g, a} target_probs[n,a]
               lp[g]  = (log(Z[g]+eps) - Lin[g]) / (V[g]+eps)
  out = loss_val + mean(lp)

Sharding: data-parallel over graphs. Each of the 8 cores owns 512 whole
graphs = 262144 nodes. Per-core layout: the (262144, 10) node-major shard
is viewed as (128, 20480) so partition p holds 4 whole graphs
(4 x 512 nodes x 10 actions = 20480 contiguous floats); a graph is a
5120-wide contiguous block of the free axis, split across column tiles.

The host casts logits/probs to bf16 before upload (STREAM_BF16): this
halves the HBM stream (the original binding constraint) to ~10.5 MB/core,
which puts all 8 cores together right at the chip HBM roofline
(~29us/core). All accumulation stays f32, so only the bf16 input rounding
propagates: measured rel err 2.3e-07 (gate is 2e-2).

Per column tile (128 x w): ACT does exp with fused accumulate (Z) plus
copy-accumulates (V) for the leading columns; DVE does one fused
multiply+reduce via scalar_tensor_tensor (Lin) plus reduce_sum (V) for
the V_TAIL_DVE trailing columns. With bf16 the two engines are jointly
reduction-bound (exp + V + Lin = 3 passes over the data at ~1.07 ns/elem,
~33us per engine) and finish just after the stream. The last graph tapers
[2048, 1536, 1024, 512] so the post-last-byte serial chain is thin.

Epilogue: per-graph losses on (128, GPP) stats accumulate into S[128, 8];
with DIRECT_OUT the per-partition partials DMA straight out and the host
reduces (skips the PE ones-matmul + PSUM copy + extra descriptor gen).
Mode "partials" (default) avoids any collective barrier entirely.
"""

import numpy as np

B = 4096
NPG = 512
N = B * NPG
A = 10
EPS = 1e-9
M = 8  # cores

NC_NODES = N // M          # 262144 nodes per core
NC_GRAPHS = B // M         # 512 graphs per core
P = 128                    # SBUF partitions
FREE = NC_NODES * A // P   # 20480 f32 per partition
GPP = NC_GRAPHS // P       # 4 graphs per partition
GFREE = NPG * A            # 5120 f32 per graph

# Per-graph tile widths (must sum to GFREE). A thin final slice keeps the
# critical path after the last DMA byte short.
GRAPH_TILES = [2560, 2560]
TPG = len(GRAPH_TILES)
NT = GPP * TPG
# When True, the very last graph column uses widths LAST_WIDTHS so the
# final DVE/ACT ops after the last DMA byte are thin (geometric taper:
# each op after the last byte processes only the final sliver).
SPLIT_LAST = True
LAST_WIDTHS = [2048, 1536, 1024, 512]

IO_BUFS = 4
WORK_BUFS = 3

# Number of trailing columns whose V-reduce runs on DVE (reduce_sum)
# instead of ACT (Copy+accum) — balances the two engines now that bf16
# streaming makes ACT the near-bottleneck (bf16 sweep: 5 beats 2/4/7/9).
V_TAIL_DVE = 5

# When True, invert the V split: DVE reduces the FIRST V_TAIL_DVE columns
# (where it has arrival slack) and ACT copy-accumulates the tail, so the
# DVE queue is not double-loaded right at stream end.
V_FRONT = False

# Carve the FINAL N columns out of the DVE V-tail and give their V back to
# ACT: after its last exp ACT idles ~3us (table load + Ln wait) while DVE
# drains; the two smallest taper columns' copies fit in that window (they
# queue before the Ln table load, so no table thrash).
V_ACT_TAIL = 0

# Reorder the stream tail: the last graph's two logits half-tiles are
# DMA'd before the matching probs half-tiles, so exp + the Ln table load
# + LN finish before the last probs byte lands.
TAIL_REORDER = False

# Engine whose HWDGE queue issues the probs-tile DMAs ("sync" = same queue
# as logits; "tensor" = the otherwise-idle PE sequencer, doubling the
# DMA-issue paths).
PT_ENGINE = "sync"

# Columns whose DMA is issued as [0:120] + [120:128] row-splits. A sync-HWDGE
# transfer deals ceil(nrows/16) contiguous rows per SDMA engine starting at
# engine 0, so the [120,*] part leaves engine 15 idle and the [8,*] part rides
# engines 0-7. SDMA engine 15 is ~20% slower on core 0 whenever the profiler
# is attached (the slow mode of the bimodal exec distribution); skipping it
# for the last graph's columns drops e15 to ~75% of the per-engine byte load
# so its backlog drains before the other engines finish. A/B (7 interleaved
# reps each): split 81.0-87.0us, never fast; baseline 68.7-81.7 — the extra
# transfers slow the whole stream more than the e15 skip helps. Keep False.
E15_SPLIT_TAIL = False

# Number of leading columns whose DMAs issue from the scalar (Act) HWDGE
# instead of sync: the Act sequencer's preamble runs in parallel with
# Sync's, so the first descriptors generate ~1.5us earlier.
FIRST_SCALAR_COLS = 0

# Band-skew layout: rows 120-127 hold only 2 graphs, so the columns of graph
# slots 2 and 3 are [0:120]-row transfers that put ZERO descriptors on SDMA
# engine 15 (sync HWDGE deals ceil(nrows/16) contiguous rows per engine from
# engine 0; engine 15 is ~20% slower on core 0 whenever the NTFF profiler
# runs — the slow mode of the bimodal exec distribution). The 16 displaced
# graphs ride as separate [16, 5120] band tensors — ONE graph per partition
# row, ONE 20.5KB descriptor per SDMA engine (the per-descriptor fixed cost
# is ~300ns, so band shape must minimize descriptor count, which is what
# sank the [128, 320] variant). Two PE matmuls against a host-provided
# selection matrix W[16, 128] scatter the band's per-graph sums onto rows
# 120-127, into two extra stats columns that the slot-2/slot-3 reduces fold
# in with no extra tail ops. Engine 15 carries ~52% of the other engines'
# bytes, so it never gates the stream even when 25% slow.
BAND_SKEW = False
SKEW_LO = 120       # rows >= SKEW_LO carry 2 graphs under BAND_SKEW
NBAND = 16          # displaced graphs (one band-tensor row each)

# False: partials leave via a PE ones-matmul -> PSUM[1,8] -> SBUF -> DRAM.
# True: DMA the per-partition S[128,8] straight out; host reduces. With the
# bf16 tapered tail this skips the matmul+PSUM-copy+descriptor-gen chain:
# A/B 59.9 vs 60.4 / 51.5 vs 52+ us. Keep True.
DIRECT_OUT = True

# "lfirst": stream the whole logits shard first (kept resident in SBUF,
# exp/log(Z) finish mid-stream), then stream probs; the post-last-byte
# chain shrinks to one half-width stt + copy + the scalar epilogue.
LFIRST = False

# Stream logits/probs as bf16 (host casts before upload): halves the HBM
# bytes of the DMA-bound stream and doubles DVE elementwise throughput.
# All accumulation stays f32 (ACT/DVE accumulators), so only the 0.2%
# input rounding propagates: measured rel err ~1e-4 vs the 2e-2 gate.
# vals/tvals stay f32 (tiny, and keeps loss_val exact).
STREAM_BF16 = True


def set_params(graph_tiles=None, io_bufs=None, work_bufs=None, mode=None,
               split_last=None, lfirst=None, direct_out=None,
               pt_engine=None, tail_reorder=None, v_tail_dve=None,
               e15_split_tail=None, first_scalar_cols=None, band_skew=None,
               stream_bf16=None, last_widths=None, v_front=None,
               v_act_tail=None):
    """Tweak build knobs (test harness only); clears the build cache."""
    global GRAPH_TILES, TPG, NT, IO_BUFS, WORK_BUFS, MODE, SPLIT_LAST, LFIRST
    global DIRECT_OUT, PT_ENGINE, TAIL_REORDER, V_TAIL_DVE, E15_SPLIT_TAIL
    global FIRST_SCALAR_COLS, BAND_SKEW, STREAM_BF16, LAST_WIDTHS, V_FRONT
    global V_ACT_TAIL
    if v_act_tail is not None:
        V_ACT_TAIL = v_act_tail
    if v_front is not None:
        V_FRONT = v_front
    if last_widths is not None:
        assert sum(last_widths) == GFREE
        LAST_WIDTHS = list(last_widths)
    if stream_bf16 is not None:
        STREAM_BF16 = stream_bf16
    if band_skew is not None:
        BAND_SKEW = band_skew
    if first_scalar_cols is not None:
        FIRST_SCALAR_COLS = first_scalar_cols
    if e15_split_tail is not None:
        E15_SPLIT_TAIL = e15_split_tail
    if v_tail_dve is not None:
        V_TAIL_DVE = v_tail_dve
    if tail_reorder is not None:
        TAIL_REORDER = tail_reorder
    if pt_engine is not None:
        PT_ENGINE = pt_engine
    if split_last is not None:
        SPLIT_LAST = split_last
    if lfirst is not None:
        LFIRST = lfirst
    if direct_out is not None:
        DIRECT_OUT = direct_out
    if graph_tiles is not None:
        assert sum(graph_tiles) == GFREE
        GRAPH_TILES = list(graph_tiles)
        TPG = len(GRAPH_TILES)
        NT = GPP * TPG
    if io_bufs is not None:
        IO_BUFS = io_bufs
    if work_bufs is not None:
        WORK_BUFS = work_bufs
    if mode is not None:
        MODE = mode
    _CACHE.clear()

MODE = "partials"

_CACHE = {}


def _build(mode):
    import concourse.bacc as bacc
    import concourse.mybir as mybir
    import concourse.tile as tile

    f32 = mybir.dt.float32
    sdt = mybir.dt.bfloat16 if STREAM_BF16 else f32
    Alu = mybir.AluOpType
    Act = mybir.ActivationFunctionType
    AX = mybir.AxisListType.X

    nc = bacc.Bacc("TRN2", target_bir_lowering=False, debug=False,
                   num_devices=M)

    logits = nc.dram_tensor("logits", [P, FREE], sdt, kind="ExternalInput")
    probs = nc.dram_tensor("probs", [P, FREE], sdt, kind="ExternalInput")
    vals = nc.dram_tensor("vals", [P, GPP], f32, kind="ExternalInput")
    tvals = nc.dram_tensor("tvals", [P, GPP], f32, kind="ExternalInput")
    if BAND_SKEW:
        lband = nc.dram_tensor("lband", [NBAND, GFREE], sdt,
                               kind="ExternalInput")
        pband = nc.dram_tensor("pband", [NBAND, GFREE], sdt,
                               kind="ExternalInput")
        wsel = nc.dram_tensor("wsel", [NBAND, 2 * P], f32,
                              kind="ExternalInput")
    if mode == "allreduce" or not DIRECT_OUT:
        out = nc.dram_tensor("out", [1, 8], f32, kind="ExternalOutput")
    else:
        # per-partition partials go out directly; host does the final
        # 128x2-per-core reduction (no PE/PSUM in the graph at all)
        out = nc.dram_tensor("out", [P, 8], f32, kind="ExternalOutput")

    import contextlib

    with tile.TileContext(nc) as tc:
        with contextlib.ExitStack() as ctx:
            iop = ctx.enter_context(tc.tile_pool(name="io", bufs=IO_BUFS))
            wp = ctx.enter_context(tc.tile_pool(name="work", bufs=WORK_BUFS))
            sp = ctx.enter_context(tc.tile_pool(name="stats", bufs=1))
            if BAND_SKEW:
                bpool = ctx.enter_context(tc.tile_pool(name="band", bufs=1))
            if mode == "allreduce" or not DIRECT_OUT:
                pp = ctx.enter_context(
                    tc.tile_pool(name="psum", bufs=1, space="PSUM"))
            if mode == "allreduce":
                dp = ctx.enter_context(
                    tc.tile_pool(name="dram", bufs=1, space="DRAM"))
            # per-tile widths for each graph column; optionally split the
            # tail of the last graph for a shorter post-DMA chain
            widths = []
            for g in range(GPP):
                if SPLIT_LAST and g == GPP - 1:
                    widths += list(LAST_WIDTHS)
                else:
                    widths += list(GRAPH_TILES)
            ncols = len(widths)
            # Stats columns under BAND_SKEW: data cols for graph slots 0-2
            # keep their index, slot 3's shift up by one; two band columns
            # (slot-2 sums at index 2*TPG+2, slot-3 sums at the end) are
            # filled from PSUM mid-stream so the slot reduces fold them in.
            if BAND_SKEW:
                assert SPLIT_LAST and GPP == 4 and TPG == 2
                bcol2 = 2 * TPG + 2                     # 6
                colmap = [0, 1, 2, 3, 4, 5, 7, 8, 9]
                ncols_b = ncols + 2                     # 11; bcol3 = 10
                bcol3 = ncols_b - 1
            else:
                colmap = list(range(ncols))
                ncols_b = ncols
            Z = sp.tile([P, ncols_b], f32)
            Lin = sp.tile([P, ncols_b], f32)
            V = sp.tile([P, ncols_b], f32)

            fmax = max(GRAPH_TILES)
            offs = []
            o = 0
            for w in widths:
                offs.append(o)
                o += w

            # columns belonging to the last graph get engine-15-free DMAs
            e15_free = set(range(ncols - (TPG + (1 if SPLIT_LAST else 0)),
                                 ncols)) if E15_SPLIT_TAIL else set()
            # under BAND_SKEW, rows 120-127 have no data for graph slots
            # 2 and 3: transfer rows [0:120] only (zero bytes on SDMA
            # engine 15) and memset rows [96:128] so the full-partition
            # compute reads neutral values there
            band_cols = (set(j for j in range(ncols)
                             if offs[j] >= (GPP - 2) * GFREE)
                         if BAND_SKEW else set())

            def col_dma(dst, src_t, j, w, fill=0.0):
                eng = nc.scalar if j < FIRST_SCALAR_COLS else nc.sync
                if j in band_cols:
                    nc.gpsimd.memset(dst[96:128, :w], fill)
                    eng.dma_start(dst[0:SKEW_LO, :w],
                                  src_t[0:SKEW_LO, offs[j]:offs[j] + w])
                elif j in e15_free:
                    eng.dma_start(dst[0:120, :w],
                                  src_t[0:120, offs[j]:offs[j] + w])
                    eng.dma_start(dst[120:128, :w],
                                  src_t[120:128, offs[j]:offs[j] + w])
                else:
                    eng.dma_start(dst[:, :w],
                                  src_t[:, offs[j]:offs[j] + w])

            def emit_lt(j):
                lt = iop.tile([P, fmax], sdt, tag="lt", name=f"lt{j}")
                col_dma(lt, logits, j, widths[j], fill=-80.0)
                return lt

            # op outputs are dummies (only accum_out matters) — under
            # BAND_SKEW share one per engine to make SBUF room for the
            # band tiles (WAW ordering matches engine program order)
            if BAND_SKEW:
                act_dum = wp.tile([P, fmax], sdt, tag="actd", name="actd")
                dve_dum = wp.tile([P, fmax], sdt, tag="dved", name="dved")

            def emit_exp(j, lt):
                w = widths[j]
                c = colmap[j]
                et = (act_dum if BAND_SKEW else
                      wp.tile([P, fmax], sdt, tag="et", name=f"et{j}"))
                nc.scalar.activation(et[:, :w], lt[:, :w], Act.Exp,
                                     accum_out=Z[:, c:c + 1])

            def emit_pt_side(j, lt):
                w = widths[j]
                c = colmap[j]
                pt = iop.tile([P, fmax], sdt, tag="pt", name=f"pt{j}")
                col_dma(pt, probs, j, w)
                on_dve = (j < V_TAIL_DVE) if V_FRONT else (
                    ncols - V_TAIL_DVE <= j < ncols - V_ACT_TAIL)
                if on_dve:
                    nc.vector.reduce_sum(V[:, c:c + 1], pt[:, :w], axis=AX)
                else:
                    cp = (act_dum if BAND_SKEW else
                          wp.tile([P, fmax], sdt, tag="cp", name=f"cp{j}"))
                    nc.scalar.activation(cp[:, :w], pt[:, :w], Act.Copy,
                                         accum_out=V[:, c:c + 1])
                prod = (dve_dum if BAND_SKEW else
                        wp.tile([P, fmax], sdt, tag="prod", name=f"prod{j}"))
                nc.vector.scalar_tensor_tensor(
                    out=prod[:, :w], in0=lt[:, :w], scalar=1.0,
                    in1=pt[:, :w], op0=Alu.mult, op1=Alu.mult,
                    accum_out=Lin[:, c:c + 1])

            split_tail = SPLIT_LAST or BAND_SKEW
            nuni = (GPP - 1) * TPG if split_tail else GPP * TPG
            Zg = sp.tile([P, GPP], f32)
            Lg = sp.tile([P, GPP], f32)
            Vg = sp.tile([P, GPP], f32)
            zp = sp.tile([P, GPP], f32)
            logz = sp.tile([P, GPP], f32)

            def emit_logz():
                if BAND_SKEW:
                    nc.vector.reduce_sum(
                        Zg[:, 0:2],
                        Z[:, 0:4].rearrange("p (g t) -> p g t", t=TPG),
                        axis=AX)
                    nc.vector.reduce_sum(
                        Zg[:, 2:3], Z[:, 4:bcol2 + 1], axis=AX)
                    nc.vector.reduce_sum(
                        Zg[:, 3:4], Z[:, bcol2 + 1:ncols_b], axis=AX)
                elif split_tail:
                    nc.vector.reduce_sum(
                        Zg[:, 0:GPP - 1],
                        Z[:, 0:nuni].rearrange("p (g t) -> p g t", t=TPG),
                        axis=AX)
                    nc.vector.reduce_sum(
                        Zg[:, GPP - 1:GPP], Z[:, nuni:ncols_b], axis=AX)
                else:
                    nc.vector.reduce_sum(
                        Zg[:, :],
                        Z[:, :].rearrange("p (g t) -> p g t", t=TPG),
                        axis=AX)
                nc.vector.tensor_scalar_add(zp[:, :], Zg[:, :], EPS)
                nc.scalar.activation(logz[:, :], zp[:, :], Act.Ln)

            def emit_band():
                # host-filled selection pair: cols 0-127 select band graphs
                # 0-7 (slot 2 of rows 120-127), cols 128-255 graphs 8-15
                # (slot 3); zero rows kill the other group, keeping both
                # matmuls at partition base 0
                wt = sp.tile([NBAND, 2 * P], f32)
                nc.sync.dma_start(wt[:, :], wsel[:, :])
                bl = bpool.tile([NBAND, GFREE], f32, tag="bl", name="bl")
                nc.sync.dma_start(bl[:, :], lband[:, :])
                bp = bpool.tile([NBAND, GFREE], f32, tag="bp", name="bp")
                nc.sync.dma_start(bp[:, :], pband[:, :])
                BS = sp.tile([NBAND, 3], f32)
                bdum = bpool.tile([NBAND, GFREE], f32, tag="bdum",
                                  name="bdum")
                nc.scalar.activation(bdum[:, :], bl[:, :], Act.Exp,
                                     accum_out=BS[:, 0:1])
                nc.scalar.activation(bdum[:, :], bp[:, :], Act.Copy,
                                     accum_out=BS[:, 1:2])
                nc.vector.scalar_tensor_tensor(
                    out=bdum[:, :], in0=bl[:, :], scalar=1.0, in1=bp[:, :],
                    op0=Alu.mult, op1=Alu.mult, accum_out=BS[:, 2:3])
                # per-graph scatter onto rows 120-127: cols 0-2 = slot-2
                # band sums (Z, V, Lin), cols 3-5 = slot-3
                pf = pp.tile([P, 6], f32)
                nc.tensor.matmul(pf[:, 0:3], wt[:, 0:P], BS[:, :],
                                 start=True, stop=True)
                nc.tensor.matmul(pf[:, 3:6], wt[:, P:2 * P], BS[:, :],
                                 start=True, stop=True)
                # rows 0-119 of pf are zero, so the copies only affect the
                # skew rows' stat slots
                nc.vector.tensor_copy(Z[:, bcol2:bcol2 + 1], pf[:, 0:1])
                nc.vector.tensor_copy(V[:, bcol2:bcol2 + 1], pf[:, 1:2])
                nc.vector.tensor_copy(Lin[:, bcol2:bcol2 + 1], pf[:, 2:3])
                nc.vector.tensor_copy(Z[:, bcol3:bcol3 + 1], pf[:, 3:4])
                nc.vector.tensor_copy(V[:, bcol3:bcol3 + 1], pf[:, 4:5])
                nc.vector.tensor_copy(Lin[:, bcol3:bcol3 + 1], pf[:, 5:6])

            late = 2 if (TAIL_REORDER and SPLIT_LAST) else 0
            lts = {}
            for j in range(ncols - late):
                lts[j] = emit_lt(j)
                emit_exp(j, lts[j])
                emit_pt_side(j, lts[j])
                if BAND_SKEW and j == 0:
                    emit_band()
            if late:
                for j in range(ncols - late, ncols):
                    lts[j] = emit_lt(j)
                    emit_exp(j, lts[j])
                emit_logz()
                for j in range(ncols - late, ncols):
                    emit_pt_side(j, lts[j])
            else:
                emit_logz()

            # remaining per-graph sums
            for src, dst in ((Lin, Lg), (V, Vg)):
                if BAND_SKEW:
                    nc.vector.reduce_sum(
                        dst[:, 0:2],
                        src[:, 0:4].rearrange("p (g t) -> p g t", t=TPG),
                        axis=AX)
                    nc.vector.reduce_sum(
                        dst[:, 2:3], src[:, 4:bcol2 + 1], axis=AX)
                    nc.vector.reduce_sum(
                        dst[:, 3:4], src[:, bcol2 + 1:ncols_b], axis=AX)
                elif split_tail:
                    nc.vector.reduce_sum(
                        dst[:, 0:GPP - 1],
                        src[:, 0:nuni].rearrange("p (g t) -> p g t", t=TPG),
                        axis=AX)
                    nc.vector.reduce_sum(
                        dst[:, GPP - 1:GPP], src[:, nuni:ncols_b], axis=AX)
                else:
                    nc.vector.reduce_sum(
                        dst[:, :],
                        src[:, :].rearrange("p (g t) -> p g t", t=TPG),
                        axis=AX)

            num = sp.tile([P, GPP], f32)
            nc.vector.tensor_sub(num[:, :], logz[:, :], Lg[:, :])
            den = sp.tile([P, GPP], f32)
            nc.vector.tensor_scalar_add(den[:, :], Vg[:, :], EPS)
            rec = sp.tile([P, GPP], f32)
            nc.vector.reciprocal(rec[:, :], den[:, :])

            # S[:,0] = per-partition policy sum, S[:,1] = value-sq sum
            S = sp.tile([P, 8], f32)
            nc.gpsimd.memset(S[:, :], 0.0)
            lp = sp.tile([P, GPP], f32)
            nc.vector.scalar_tensor_tensor(
                out=lp[:, :], in0=num[:, :], scalar=1.0, in1=rec[:, :],
                op0=Alu.mult, op1=Alu.mult, accum_out=S[:, 0:1])

            vt = sp.tile([P, GPP], f32)
            tt = sp.tile([P, GPP], f32)
            nc.sync.dma_start(vt[:, :], vals[:, :])
            nc.sync.dma_start(tt[:, :], tvals[:, :])
            d = sp.tile([P, GPP], f32)
            nc.vector.tensor_sub(d[:, :], vt[:, :], tt[:, :])
            d2 = sp.tile([P, GPP], f32)
            nc.vector.scalar_tensor_tensor(
                out=d2[:, :], in0=d[:, :], scalar=1.0, in1=d[:, :],
                op0=Alu.mult, op1=Alu.mult, accum_out=S[:, 1:2])

            if mode == "allreduce" or not DIRECT_OUT:
                # cross-partition sum via matmul with a ones vector
                ones = sp.tile([P, 1], f32)
                nc.gpsimd.memset(ones[:, :], 1.0)
                ps = pp.tile([1, 8], f32)
                nc.tensor.matmul(ps[:, :], ones[:, :], S[:, :],
                                 start=True, stop=True)
                red = sp.tile([1, 8], f32)
                nc.vector.tensor_copy(red[:, :], ps[:, :])
            if mode == "allreduce":
                cin = dp.tile([1, 8], f32)
                cout = dp.tile([1, 8], f32)
                nc.sync.dma_start(cin[:, :], red[:, :])
                nc.gpsimd.collective_compute(
                    "AllReduce", Alu.add,
                    replica_groups=[list(range(M))],
                    ins=[cin[:, :].opt()],
                    outs=[cout[:, :].opt()])
                red2 = sp.tile([1, 8], f32)
                nc.sync.dma_start(red2[:, :], cout[:, :])
                # out = (sum_policy + sum_val) / B
                dummy = sp.tile([1, 2], f32)
                fin = sp.tile([1, 8], f32)
                nc.gpsimd.memset(fin[:, :], 0.0)
                nc.scalar.activation(dummy[:, :], red2[:, 0:2], Act.Copy,
                                     scale=1.0 / B, accum_out=fin[:, 0:1])
                nc.sync.dma_start(out[:, :], fin[:, :])
            elif DIRECT_OUT:
                nc.sync.dma_start(out[:, :], S[:, :])
            else:
                nc.sync.dma_start(out[:, :], red[:, :])

    nc.compile()
    return nc


def _build_lfirst(mode):
    import concourse.bacc as bacc
    import concourse.mybir as mybir
    import concourse.tile as tile

    f32 = mybir.dt.float32
    Alu = mybir.AluOpType
    Act = mybir.ActivationFunctionType
    AX = mybir.AxisListType.X

    nc = bacc.Bacc("TRN2", target_bir_lowering=False, debug=False,
                   num_devices=M)

    logits = nc.dram_tensor("logits", [P, FREE], f32, kind="ExternalInput")
    probs = nc.dram_tensor("probs", [P, FREE], f32, kind="ExternalInput")
    vals = nc.dram_tensor("vals", [P, GPP], f32, kind="ExternalInput")
    tvals = nc.dram_tensor("tvals", [P, GPP], f32, kind="ExternalInput")
    out = nc.dram_tensor("out", [1, 8], f32, kind="ExternalOutput")

    LW = 2560                    # logits tile width
    LNT = FREE // LW             # 8 resident logits tiles
    # probs widths: uniform except the last graph's tail is split in half
    pw = [LW] * (LNT - 1) + [LW // 2, LW // 2]
    PNT = len(pw)                # 9
    LEAD = 3                     # logits tiles ahead of probs in the stream

    with tile.TileContext(nc) as tc:
        with (
            tc.tile_pool(name="lres", bufs=LNT) as lrp,
            tc.tile_pool(name="pio", bufs=IO_BUFS) as pip_,
            tc.tile_pool(name="work", bufs=WORK_BUFS) as wp,
            tc.tile_pool(name="stats", bufs=1) as sp,
            tc.tile_pool(name="psum", bufs=1, space="PSUM") as pp,
        ):
            Z = sp.tile([P, LNT], f32)
            V = sp.tile([P, PNT], f32)
            Lin = sp.tile([P, PNT], f32)

            ltiles = [lrp.tile([P, LW], f32, tag="lt", name=f"lt{j}")
                      for j in range(LNT)]

            def emit_logits(j):
                nc.sync.dma_start(ltiles[j][:, :],
                                  logits[:, j * LW:(j + 1) * LW])
                et = wp.tile([P, LW], f32, tag="et", name=f"et{j}")
                nc.scalar.activation(et[:, :], ltiles[j][:, :], Act.Exp,
                                     accum_out=Z[:, j:j + 1])

            poff = [0]

            def emit_probs(j):
                w = pw[j]
                off = poff[0]
                pt = pip_.tile([P, LW], f32, tag="pt", name=f"pt{j}")
                nc.sync.dma_start(pt[:, :w], probs[:, off:off + w])
                lsrc = ltiles[off // LW][:, off % LW:off % LW + w]
                cp = wp.tile([P, LW], f32, tag="cp", name=f"cp{j}")
                nc.scalar.activation(cp[:, :w], pt[:, :w], Act.Copy,
                                     accum_out=V[:, j:j + 1])
                prod = wp.tile([P, LW], f32, tag="prod", name=f"prod{j}")
                nc.vector.scalar_tensor_tensor(
                    out=prod[:, :w], in0=lsrc, scalar=1.0, in1=pt[:, :w],
                    op0=Alu.mult, op1=Alu.mult,
                    accum_out=Lin[:, j:j + 1])
                poff[0] += w

            # interleaved stream: logits LEAD tiles ahead so exp/log(Z)
            # finish before the probs stream ends
            li = pi = 0
            for j in range(LEAD):
                emit_logits(li)
                li += 1
            while li < LNT:
                emit_probs(pi)
                pi += 1
                emit_logits(li)
                li += 1
            # log(Z+eps) per graph — scheduled right after the last exp,
            # well before the stream ends
            Zg = sp.tile([P, GPP], f32)
            nc.vector.reduce_sum(
                Zg[:, :], Z[:, :].rearrange("p (g t) -> p g t", t=2),
                axis=AX)
            zp = sp.tile([P, GPP], f32)
            nc.vector.tensor_scalar_add(zp[:, :], Zg[:, :], EPS)
            logz = sp.tile([P, GPP], f32)
            nc.scalar.activation(logz[:, :], zp[:, :], Act.Ln)
            while pi < PNT:
                emit_probs(pi)
                pi += 1

            # per-graph sums: graphs 0..2 from column pairs, graph 3 from
            # the last three columns
            Vg = sp.tile([P, GPP], f32)
            Lg = sp.tile([P, GPP], f32)
            for src, dst in ((V, Vg), (Lin, Lg)):
                nc.vector.reduce_sum(
                    dst[:, 0:GPP - 1],
                    src[:, 0:2 * (GPP - 1)].rearrange(
                        "p (g t) -> p g t", t=2),
                    axis=AX)
                nc.vector.reduce_sum(dst[:, GPP - 1:GPP],
                                     src[:, 2 * (GPP - 1):PNT], axis=AX)

            den = sp.tile([P, GPP], f32)
            nc.vector.tensor_scalar_add(den[:, :], Vg[:, :], EPS)
            rec = sp.tile([P, GPP], f32)
            nc.vector.reciprocal(rec[:, :], den[:, :])
            num = sp.tile([P, GPP], f32)
            nc.vector.tensor_sub(num[:, :], logz[:, :], Lg[:, :])

            S = sp.tile([P, 8], f32)
            nc.gpsimd.memset(S[:, :], 0.0)
            lp = sp.tile([P, GPP], f32)
            nc.vector.scalar_tensor_tensor(
                out=lp[:, :], in0=num[:, :], scalar=1.0, in1=rec[:, :],
                op0=Alu.mult, op1=Alu.mult, accum_out=S[:, 0:1])

            vt = sp.tile([P, GPP], f32)
            tt = sp.tile([P, GPP], f32)
            nc.sync.dma_start(vt[:, :], vals[:, :])
            nc.sync.dma_start(tt[:, :], tvals[:, :])
            d = sp.tile([P, GPP], f32)
            nc.vector.tensor_sub(d[:, :], vt[:, :], tt[:, :])
            d2 = sp.tile([P, GPP], f32)
            nc.vector.scalar_tensor_tensor(
                out=d2[:, :], in0=d[:, :], scalar=1.0, in1=d[:, :],
                op0=Alu.mult, op1=Alu.mult, accum_out=S[:, 1:2])

            ones = sp.tile([P, 1], f32)
            nc.gpsimd.memset(ones[:, :], 1.0)
            ps = pp.tile([1, 8], f32)
            nc.tensor.matmul(ps[:, :], ones[:, :], S[:, :],
                             start=True, stop=True)
            red = sp.tile([1, 8], f32)
            nc.vector.tensor_copy(red[:, :], ps[:, :])
            nc.sync.dma_start(out[:, :], red[:, :])

    nc.compile()
    return nc


def _get(mode):
    if mode not in _CACHE:
        _CACHE[mode] = (_build_lfirst(mode) if LFIRST and mode == "partials"
                        else _build(mode))
    return _CACHE[mode]


def _band_gidx():
    """graph index for (row, slot) under BAND_SKEW: rows 0-119 hold graphs
    4p..4p+3; rows 120-127 hold graphs 480+2r, 481+2r plus band graphs
    496+r (slot 2) and 504+r (slot 3)."""
    gidx = np.empty((P, GPP), np.int64)
    for p_ in range(SKEW_LO):
        gidx[p_] = np.arange(4 * p_, 4 * p_ + 4)
    for r in range(P - SKEW_LO):
        gidx[SKEW_LO + r, 0:2] = 480 + 2 * r + np.arange(2)
        gidx[SKEW_LO + r, 2] = 496 + r
        gidx[SKEW_LO + r, 3] = 504 + r
    return gidx


def _band_wsel():
    w = np.zeros((NBAND, 2 * P), np.float32)
    for g in range(8):
        w[g, SKEW_LO + g] = 1.0               # slot-2 selector
        w[8 + g, P + SKEW_LO + g] = 1.0       # slot-3 selector
    return w


def _band_shard(flat_core):
    """(NC_NODES*A,) per-core flat stream -> main [P, FREE] + band
    [NBAND, GFREE]."""
    graphs = flat_core.reshape(NC_GRAPHS, GFREE)
    main = np.zeros((P, FREE), np.float32)
    main[:SKEW_LO] = graphs[:480].reshape(SKEW_LO, 4 * GFREE)
    main[SKEW_LO:, :2 * GFREE] = graphs[480:496].reshape(8, 2 * GFREE)
    band = np.ascontiguousarray(graphs[496:])
    return main, band


def _stream_cast(a):
    if not STREAM_BF16:
        return np.ascontiguousarray(a)
    import ml_dtypes
    return np.ascontiguousarray(a.astype(ml_dtypes.bfloat16))


def _make_in_maps(logits, values, target_probs, target_vals):
    in_maps = []
    if BAND_SKEW:
        gidx = _band_gidx()
        wsel = _band_wsel()
        lg = logits.reshape(M, NC_NODES * A)
        pg = target_probs.reshape(M, NC_NODES * A)
        vg = values.reshape(M, NC_GRAPHS)
        tg = target_vals.reshape(M, NC_GRAPHS)
        for c in range(M):
            lmain, lb = _band_shard(lg[c])
            pmain, pb = _band_shard(pg[c])
            in_maps.append({
                "logits": _stream_cast(lmain),
                "probs": _stream_cast(pmain),
                "lband": _stream_cast(lb),
                "pband": _stream_cast(pb),
                "vals": np.ascontiguousarray(vg[c][gidx].astype(np.float32)),
                "tvals": np.ascontiguousarray(tg[c][gidx].astype(np.float32)),
                "wsel": wsel,
            })
        return in_maps
    lg = logits.reshape(M, P, FREE)
    pg = target_probs.reshape(M, P, FREE)
    vg = values.reshape(M, P, GPP)
    tg = target_vals.reshape(M, P, GPP)
    for c in range(M):
        in_maps.append({
            "logits": _stream_cast(lg[c]),
            "probs": _stream_cast(pg[c]),
            "vals": np.ascontiguousarray(vg[c]),
            "tvals": np.ascontiguousarray(tg[c]),
        })
    return in_maps


def _finalize(mode, results):
    if mode == "allreduce":
        return np.float32(results[0]["out"][0, 0])
    parts = np.stack([r["out"] for r in results])  # (M, P or 1, 8)
    tot = parts.sum(axis=(0, 1), dtype=np.float64)
    return np.float32((tot[0] + tot[1]) / B)


def kernel(logits, values, target_probs, target_vals, batch_counts):
    from concourse import bass_utils

    global STREAM_BF16
    if STREAM_BF16:
        try:
            import ml_dtypes  # noqa: F401
        except ImportError:
            STREAM_BF16 = False
            _CACHE.clear()

    logits = np.asarray(logits, dtype=np.float32)
    values = np.asarray(values, dtype=np.float32)
    target_probs = np.asarray(target_probs, dtype=np.float32)
    target_vals = np.asarray(target_vals, dtype=np.float32)
    batch_counts = np.asarray(batch_counts)

    if not (batch_counts.shape == (B,) and np.all(batch_counts == NPG)):
        # Non-uniform segments never occur for this problem's inputs;
        # numpy fallback keeps the contract total.
        return _kernel_numpy(logits, values, target_probs, target_vals,
                             batch_counts)

    nc = _get(MODE)
    in_maps = _make_in_maps(logits, values, target_probs, target_vals)
    last_err = None
    for _ in range(3):
        try:
            res = bass_utils.run_bass_kernel_spmd(
                nc, in_maps, core_ids=list(range(M)))
            return _finalize(MODE, res.results)
        except Exception as e:  # transient runtime/worker hiccup
            last_err = e
    import sys
    print(f"kernel: device run failed ({last_err}); numpy fallback",
          file=sys.stderr)
    return _kernel_numpy(logits, values, target_probs, target_vals,
                         batch_counts)


def _kernel_numpy(logits, values, target_probs, target_vals, batch_counts):
    counts = batch_counts.astype(np.int64)
    b = counts.shape[0]
    idx = np.repeat(np.arange(b), counts)
    loss_val = np.mean((values - target_vals) ** 2, dtype=np.float32)
    probs_sum = target_probs.sum(axis=1)
    lin = (target_probs * logits).sum(axis=1)
    ex = np.exp(logits).sum(axis=1)
    vc = np.zeros(b, np.float32)
    lg = np.zeros(b, np.float32)
    zg = np.zeros(b, np.float32)
    np.add.at(vc, idx, probs_sum)
    np.add.at(lg, idx, lin)
    np.add.at(zg, idx, ex)
    lp = (np.log(zg + EPS) - lg) / (vc + EPS)
    return np.float32(loss_val + lp.mean())

